# revision 1
# baseline (speedup 1.0000x reference)
"""Trainium2 Bass kernel: AlgebraicTriangulation (softargmax keypoints + DLT).

Contract: kernel(**inputs) takes FULL inputs (as in reference.setup_inputs())
and returns the FULL (64, 17, 3) output. Internally shards batch across 8
NeuronCores (pure data parallel), runs one Bass/Tile program per core via
run_bass_kernel_spmd, and reassembles on host.

Algorithm per core (8 batches = 544 heatmap rows of 96x96):
  - Each heatmap row is split into 4 quarters of 2304 elements; 2176
    quarter-units = 17 tiles of (128p, 2304f).
  - DVE `max` + `max_index` give the top-8 values+positions per quarter.
    With INV_TEMP=100 softmax mass beyond the top-8-per-quarter is < 1e-9
    relative, so the softargmax over the 32 candidates/row is exact to f32.
  - Candidates are regrouped through a DRAM scratch roundtrip into a
    (68p, 8slot, 4q, 8cand) layout; exp/weighted sums give keypoints.
  - DLT: build M = A^T A per (batch, joint) problem (136/core, laid out as
    (68p, 2slot)), run 3 sweeps of parallel-pairs cyclic Jacobi, take the
    eigenvector of the smallest eigenvalue, dehomogenize.
"""
import numpy as np
from contextlib import ExitStack

import concourse.bass as bass
import concourse.tile as tile
from concourse import bacc, mybir
from concourse.bass_utils import run_bass_kernel_spmd

f32 = mybir.dt.float32
u32 = mybir.dt.uint32
i32 = mybir.dt.int32
AF = mybir.ActivationFunctionType
ALU = mybir.AluOpType
AX = mybir.AxisListType

B, C, J, HH, WH = 64, 4, 17, 96, 96
NCORES = 8
BB = B // NCORES            # 8 local batches per core
R = BB * C * J              # 544 heatmap rows per core
NQ = 4                      # quarters per heatmap
QF = (HH * WH) // NQ        # 2304
QH = HH // NQ               # 24 rows of the heatmap per quarter
U = R * NQ                  # 2176 quarter-units
NT = U // 128               # 17 tiles
P2 = (BB // 2) * J          # 68 partitions for row/DLT stages
S8 = 8                      # fold slots: s = (bb//4)*4 + c
INV_TEMP = 100.0
SWEEPS = 3

# Jacobi parallel-pair rounds: ((p1,q1),(p2,q2)) disjoint.
ROUNDS = [((0, 1), (2, 3)), ((0, 2), (1, 3)), ((0, 3), (1, 2))]


def _build_nc(scale_x: float, scale_y: float):
    nc = bacc.Bacc(
        "TRN2", target_bir_lowering=False, debug=False,
        enable_asserts=False, num_devices=NCORES,
    )
    hm = nc.dram_tensor("hm", [R, HH * WH], f32, kind="ExternalInput").ap()
    pm = nc.dram_tensor("pm", [P2, 2 * C * 12], f32, kind="ExternalInput").ap()
    cf = nc.dram_tensor("cf", [P2, S8], f32, kind="ExternalInput").ap()
    out = nc.dram_tensor("out", [P2, 2, 3], f32, kind="ExternalOutput").ap()

    with tile.TileContext(nc) as tc, ExitStack() as ctx:
        _body(ctx, tc, nc, hm, pm, cf, out, scale_x, scale_y)
    nc.compile()
    return nc


def _body(ctx, tc, nc, hm, pm, cf, out, scale_x, scale_y):
    xpool = ctx.enter_context(tc.tile_pool(name="x", bufs=4))
    pers = ctx.enter_context(tc.tile_pool(name="pers", bufs=1))
    tmp = ctx.enter_context(tc.tile_pool(name="tmp", bufs=2))
    dpool = ctx.enter_context(tc.tile_pool(name="scratch", bufs=1, space="DRAM"))

    # ---------------- Phase A: per-quarter top-8 ----------------
    units = hm.rearrange("r (a b) -> (r a) b", a=NQ)          # (2176, 2304)
    sv = pers.tile([128, NT, 8], f32, tag="sv", name="sv")               # top-8 values
    si = pers.tile([128, NT, 8], u32, tag="si", name="si")               # top-8 indices
    sv_d = dpool.tile([U, 8], f32, tag="sv_d", name="sv_d")
    si_d = dpool.tile([U, 8], u32, tag="si_d", name="si_d")
    for t in range(NT):
        x = xpool.tile([128, QF], f32, tag="x", name="x")
        nc.sync.dma_start(x[:], units[t * 128:(t + 1) * 128, :])
        nc.vector.max(sv[:, t, :], x[:])
        nc.vector.max_index(si[:, t, :], sv[:, t, :], x[:])
        # stream stats out as soon as they're ready (unit u = 128*t + p)
        nc.sync.dma_start(sv_d[:][128 * t:128 * (t + 1), :], sv[:, t, :])
        nc.sync.dma_start(si_d[:][128 * t:128 * (t + 1), :], si[:, t, :])

    # ---------------- Phase B: regroup via DRAM scratch ----------------
    # reload merged: u = 272*s + 4*p' + q
    mv = pers.tile([P2, S8, NQ, 8], f32, tag="mv", name="mv")
    mi = pers.tile([P2, S8, NQ, 8], u32, tag="mi", name="mi")
    nc.sync.dma_start(mv[:], sv_d[:].rearrange("(s p q) c -> p s q c", s=S8, p=P2))
    nc.sync.dma_start(mi[:], si_d[:].rearrange("(s p q) c -> p s q c", s=S8, p=P2))

    SH = [P2, S8, NQ, 8]

    def T(tag, shape=None, dtype=f32, pool=tmp):
        return pool.tile(shape or SH, dtype, tag=tag, name=tag)

    # ---------------- Phase C: softargmax over 32 candidates/row ----------------
    m_row = T("m_row", [P2, S8])
    nc.vector.tensor_reduce(m_row[:], mv[:], axis=AX.XY, op=ALU.max)
    d_ = T("d_")
    nc.any.tensor_sub(d_[:], mv[:], m_row[:].unsqueeze(2).unsqueeze(3).broadcast_to(SH))
    w = T("w")
    nc.scalar.activation(w[:], d_[:], AF.Exp, bias=0.0, scale=INV_TEMP)

    fi = T("fi")
    nc.vector.tensor_copy(fi[:], mi[:])                        # u32 -> f32 (exact, < 2^24)
    # y = floor(fi/96) via one f32->i32 convert plus two branchless range
    # corrections; exact under BOTH convert rounding modes (HW rounds to
    # nearest, CoreSim truncates), since y0 is off by at most one either way.
    t1 = T("div_t1")
    nc.any.tensor_scalar_mul(t1[:], fi[:], 1.0 / WH)
    yi = T("div_yi", dtype=i32)
    nc.vector.tensor_copy(yi[:], t1[:])
    yf = T("div_yf")
    nc.vector.tensor_copy(yf[:], yi[:])
    r0 = T("div_r0")
    nc.vector.scalar_tensor_tensor(r0[:], yf[:], -float(WH), fi[:],
                                   op0=ALU.mult, op1=ALU.add)
    bn = T("div_bn")
    nc.vector.tensor_scalar(bn[:], r0[:], 0.0, None, op0=ALU.is_lt)
    y1 = T("div_y1")
    nc.vector.scalar_tensor_tensor(y1[:], bn[:], -1.0, yf[:],
                                   op0=ALU.mult, op1=ALU.add)
    r1 = T("div_r1")
    nc.vector.scalar_tensor_tensor(r1[:], bn[:], float(WH), r0[:],
                                   op0=ALU.mult, op1=ALU.add)
    bh = T("div_bh")
    nc.vector.tensor_scalar(bh[:], r1[:], float(WH), None, op0=ALU.is_ge)
    yl = T("div_y2")
    nc.vector.scalar_tensor_tensor(yl[:], bh[:], 1.0, y1[:],
                                   op0=ALU.mult, op1=ALU.add)
    xg = T("div_x")
    nc.vector.scalar_tensor_tensor(xg[:], bh[:], -float(WH), r1[:],
                                   op0=ALU.mult, op1=ALU.add)
    # quarter offset 24*q
    qoff_i = pers.tile(SH, i32, tag="qoff_i", name="qoff_i")
    nc.gpsimd.iota(qoff_i[:], pattern=[[0, S8], [QH, NQ], [0, 8]], base=0,
                   channel_multiplier=0)
    qoff = pers.tile(SH, f32, tag="qoff", name="qoff")
    nc.vector.tensor_copy(qoff[:], qoff_i[:])
    yg = T("yg")
    nc.any.tensor_add(yg[:], yl[:], qoff[:])

    wx = T("wx")
    nc.any.tensor_mul(wx[:], w[:], xg[:])
    wy = T("wy")
    nc.any.tensor_mul(wy[:], w[:], yg[:])
    kxn = T("kxn", [P2, S8])
    nc.vector.tensor_reduce(kxn[:], wx[:], axis=AX.XY, op=ALU.add)
    kyn = T("kyn", [P2, S8])
    nc.vector.tensor_reduce(kyn[:], wy[:], axis=AX.XY, op=ALU.add)
    den = T("den", [P2, S8])
    nc.vector.tensor_reduce(den[:], w[:], axis=AX.XY, op=ALU.add)
    rden = T("rden", [P2, S8])
    nc.vector.reciprocal(rden[:], den[:])
    kpt = pers.tile([P2, S8, 2], f32, tag="kpt", name="kpt")
    nc.vector.scalar_tensor_tensor(kpt[:, :, 0], kxn[:], scale_x, rden[:],
                                   op0=ALU.mult, op1=ALU.mult)
    nc.vector.scalar_tensor_tensor(kpt[:, :, 1], kyn[:], scale_y, rden[:],
                                   op0=ALU.mult, op1=ALU.mult)

    # ---------------- Phase D: DLT ----------------
    pmt = pers.tile([P2, 2 * C * 12], f32, tag="pmt", name="pmt")
    nc.sync.dma_start(pmt[:], pm)
    cft = pers.tile([P2, S8], f32, tag="cft", name="cft")
    nc.sync.dma_start(cft[:], cf)

    # (b, c) merged into one free dim bc = b*4+c so every AP has <= 3 free dims
    pmr = pmt[:].rearrange("p (bc r k) -> p bc r k", r=3, k=4)              # (68,8,3,4)
    ASH = [P2, 2 * C, 2, 4]
    a0 = T("a0", ASH)
    nc.any.tensor_mul(a0[:], pmr[:, :, 2, :].unsqueeze(2).broadcast_to(ASH),
                      kpt[:].unsqueeze(3).broadcast_to(ASH))
    a1 = T("a1", ASH)
    nc.any.tensor_sub(a1[:], a0[:], pmr[:, :, 0:2, :])
    At = T("At", ASH)
    nc.any.tensor_mul(At[:], a1[:],
                      cft[:].unsqueeze(2).unsqueeze(3).broadcast_to(ASH))

    # M = A^T A per b-slot: prod[p,k,l,r] = A[p,r,k]*A[p,r,l], reduce over r
    Mt = pers.tile([P2, 2, 4, 4], f32, tag="Mt", name="Mt")
    for b in range(2):
        Ab = At[:, C * b:C * (b + 1), :, :].rearrange("p c i k -> p (c i) k")
        Akr = Ab.transpose([0, 2, 1])                                  # (68,4,8)
        PSH = [P2, 4, 4, 8]
        prod = T(f"prod{b}", PSH)
        nc.any.tensor_mul(prod[:], Akr.unsqueeze(2).broadcast_to(PSH),
                          Akr.unsqueeze(1).broadcast_to(PSH))
        nc.vector.tensor_reduce(Mt[:, b, :, :], prod[:], axis=AX.X, op=ALU.add)

    Mf = Mt[:].rearrange("p b k l -> p b (k l)")                       # (68,2,16)
    tr_ = T("tr", [P2, 2])
    nc.vector.tensor_reduce(tr_[:], Mf[:, :, 0:16:5], axis=AX.X, op=ALU.add)
    rtr = T("rtr", [P2, 2])
    nc.vector.reciprocal(rtr[:], tr_[:])
    Mn = pers.tile([P2, 2, 4, 4], f32, tag="Mn")
    nc.any.tensor_mul(Mn[:].rearrange("p b k l -> p b (k l)"), Mf,
                      rtr[:].unsqueeze(2).broadcast_to([P2, 2, 16]))

    eps_t = pers.tile([P2, 1], f32, tag="eps_t", name="eps_t")
    nc.vector.memset(eps_t[:], 1e-35)

    Vt = pers.tile([P2, 2, 4, 4], f32, tag="Vt")
    Vf = Vt[:].rearrange("p b i k -> p b (i k)")
    nc.vector.memset(Vt[:], 0.0)
    nc.vector.memset(Vf[:, :, 0:16:5], 1.0)

    Mnf = Mn[:].rearrange("p b k l -> p b (k l)")
    ASH2 = [P2, 2, 2]
    BSH = [P2, 2, 2, 4]

    def jacobi_round(rp, last=False, fillers=(None, None)):
        (p1, q1), (p2, q2) = rp
        dg = lambda k: 5 * k
        of = lambda k, l: 4 * k + l

        def sl2(a, b):
            st = b - a
            if st > 0:
                return Mnf[:, :, a:b + 1:st]
            return Mnf[:, :, a:b - 1:st]

        app = sl2(dg(p1), dg(p2))
        aqq = sl2(dg(q1), dg(q2))
        apq = sl2(of(p1, q1), of(p2, q2))

        dd = T("j_dd", ASH2)
        nc.any.tensor_sub(dd[:], aqq, app)
        aq2 = T("j_aq2", ASH2)
        nc.any.tensor_mul(aq2[:], apq, apq)
        d2 = T("j_d2", ASH2)
        nc.any.tensor_mul(d2[:], dd[:], dd[:])
        sg = T("j_sg", ASH2)
        nc.vector.tensor_scalar(sg[:], dd[:], 0.0, None, op0=ALU.is_ge)
        sg1 = T("j_sg1", ASH2)
        nc.vector.tensor_scalar(sg1[:], sg[:], 2.0, -1.0, op0=ALU.mult, op1=ALU.add)
        h2 = T("j_h2", ASH2)
        nc.vector.scalar_tensor_tensor(h2[:], aq2[:], 4.0, d2[:],
                                       op0=ALU.mult, op1=ALU.add)
        # h = sqrt(dd^2 + 4 apq^2 + eps); eps via the sqrt bias keeps h > 0
        hh = T("j_hh", ASH2)
        nc.scalar.activation(hh[:], h2[:], AF.Sqrt, bias=eps_t[:])
        ab = T("j_ab", ASH2)
        nc.any.tensor_mul(ab[:], sg1[:], dd[:])
        if fillers[0] is not None:
            fillers[0]()          # fills the DVE wait on the ACT sqrt
        rh = T("j_rh", ASH2)
        nc.vector.reciprocal(rh[:], hh[:])
        gam = T("j_gam", ASH2)
        nc.any.tensor_mul(gam[:], ab[:], rh[:])
        s1 = T("j_s1", ASH2)
        nc.any.tensor_mul(s1[:], apq, rh[:])
        cc2 = T("j_cc2", ASH2)
        nc.vector.tensor_scalar(cc2[:], gam[:], 0.5, 0.5, op0=ALU.mult, op1=ALU.add)
        s2 = T("j_s2", ASH2)
        nc.any.tensor_mul(s2[:], s1[:], sg1[:])
        # c = sqrt((1+gamma)/2); s = (apq/h)/c * sign(dd):  c^2+s^2 == 1 to
        # machine precision because s inherits 1/c (orthogonality-safe even
        # with a sloppy ACT sqrt).
        cC = T("j_c", ASH2)
        nc.scalar.sqrt(cC[:], cc2[:])
        if fillers[1] is not None:
            fillers[1]()          # fills the DVE wait on the ACT sqrt
        rc = T("j_rc", ASH2)
        nc.vector.reciprocal(rc[:], cC[:])
        sS = T("j_s", ASH2)
        nc.any.tensor_mul(sS[:], s2[:], rc[:])

        ccb = cC[:].unsqueeze(3).broadcast_to(BSH)
        ssb = sS[:].unsqueeze(3).broadcast_to(BSH)

        def rows_view(base, i1, i2):
            st = i2 - i1
            if st > 0:
                return base[:, :, i1:i2 + 1:st, :]
            return base[:, :, i1:i2 - 1:st, :]

        def rotate(base, tagp, eng):
            rP = rows_view(base, p1, p2)
            rQ = rows_view(base, q1, q2)
            t1 = T(tagp + "_1", BSH)
            eng.tensor_mul(t1[:], ccb, rP)
            t2 = T(tagp + "_2", BSH)
            eng.tensor_mul(t2[:], ssb, rQ)
            t3 = T(tagp + "_3", BSH)
            eng.tensor_mul(t3[:], ssb, rP)
            t4 = T(tagp + "_4", BSH)
            eng.tensor_mul(t4[:], ccb, rQ)
            eng.tensor_sub(rP, t1[:], t2[:])
            eng.tensor_add(rQ, t3[:], t4[:])

        if last:
            # only M's diagonal is consumed after the final rotation:
            # app' = app - t*apq, aqq' = aqq + t*apq (exact), t = s/c
            t_ = T("j_t", ASH2)
            nc.any.tensor_mul(t_[:], sS[:], rc[:])
            tapq = T("j_tapq", ASH2)
            nc.any.tensor_mul(tapq[:], t_[:], apq)
            nc.any.tensor_sub(app, app, tapq[:])
            nc.any.tensor_add(aqq, aqq, tapq[:])
        else:
            rotate(Mn[:], "j_r", nc.any)                           # rows
            rotate(Mn[:].transpose([0, 1, 3, 2]), "j_c_", nc.any)  # cols
        # V-col rotation split into two closures: the caller emits them one
        # round later, at the next round's ACT-sqrt wait points (they only
        # depend on this round's c, s).
        base = Vt[:].transpose([0, 1, 3, 2])
        rP = rows_view(base, p1, p2)
        rQ = rows_view(base, q1, q2)

        def vrot_a():
            t1 = T("j_v_1", BSH)
            nc.any.tensor_mul(t1[:], ccb, rP)
            t2 = T("j_v_2", BSH)
            nc.any.tensor_mul(t2[:], ssb, rQ)
            t3 = T("j_v_3", BSH)
            nc.any.tensor_mul(t3[:], ssb, rP)
            t4 = T("j_v_4", BSH)
            nc.any.tensor_mul(t4[:], ccb, rQ)
            nc.any.tensor_sub(rP, t1[:], t2[:])
            vrot_a.t3 = t3
            vrot_a.t4 = t4

        def vrot_b():
            nc.any.tensor_add(rQ, vrot_a.t3[:], vrot_a.t4[:])
        return (vrot_a, vrot_b)

    pending_v = (None, None)
    for s in range(SWEEPS):
        for ri, rp in enumerate(ROUNDS):
            pending_v = jacobi_round(
                rp, last=(s == SWEEPS - 1 and ri == len(ROUNDS) - 1),
                fillers=pending_v)
    pending_v[0]()
    pending_v[1]()

    # ---------------- extract smallest-eigenvalue column, dehomogenize ----------------
    diag = Mnf[:, :, 0:16:5]
    dmin = T("dmin", [P2, 2])
    nc.vector.tensor_reduce(dmin[:], diag, axis=AX.X, op=ALU.min)
    maskm = T("maskm", [P2, 2, 4])
    nc.any.tensor_tensor(maskm[:], diag, dmin[:].unsqueeze(2).broadcast_to([P2, 2, 4]),
                         op=ALU.is_equal)
    vm = T("vm", [P2, 2, 4, 4])
    nc.any.tensor_mul(vm[:], Vt[:], maskm[:].unsqueeze(2).broadcast_to([P2, 2, 4, 4]))
    vsel = T("vsel", [P2, 2, 4])
    nc.vector.tensor_reduce(vsel[:], vm[:], axis=AX.X, op=ALU.add)
    rw = T("rw", [P2, 2])
    nc.vector.reciprocal(rw[:], vsel[:, :, 3])
    o3 = T("o3", [P2, 2, 3])
    nc.any.tensor_mul(o3[:], vsel[:, :, 0:3],
                      rw[:].unsqueeze(2).broadcast_to([P2, 2, 3]))
    nc.sync.dma_start(out, o3[:])


_NC_CACHE = {}


def _get_nc(scale_x: float, scale_y: float):
    key = (scale_x, scale_y)
    if key not in _NC_CACHE:
        _NC_CACHE[key] = _build_nc(scale_x, scale_y)
    return _NC_CACHE[key]


def make_in_maps(heatmaps, projection_matrices, confidences):
    hm = np.asarray(heatmaps, dtype=np.float32)
    P = np.asarray(projection_matrices, dtype=np.float32)
    cfa = np.asarray(confidences, dtype=np.float32)
    hmv = hm.reshape(NCORES, 2, BB // 2, C, J, HH * WH)   # (core, bb2, bb4, c, j, F)
    Pv = P.reshape(NCORES, 2, BB // 2, C, 3, 4)
    cfv = cfa.reshape(NCORES, 2, BB // 2, C, J)
    in_maps = []
    for k in range(NCORES):
        hmk = np.ascontiguousarray(hmv[k].transpose(0, 2, 1, 3, 4)).reshape(R, HH * WH)
        pmk = np.ascontiguousarray(np.broadcast_to(
            Pv[k].transpose(1, 0, 2, 3, 4)[:, None], (BB // 2, J, 2, C, 3, 4)
        )).reshape(P2, 2 * C * 12)
        cfk = np.ascontiguousarray(cfv[k].transpose(1, 3, 0, 2)).reshape(P2, S8)
        in_maps.append({"hm": hmk, "pm": pmk, "cf": cfk})
    return in_maps


def assemble_out(results):
    outs = []
    for k in range(NCORES):
        o = results[k]["out"].reshape(BB // 2, J, 2, 3).transpose(2, 0, 1, 3)
        outs.append(o.reshape(BB, J, 3))
    return np.concatenate(outs, axis=0)


def kernel(heatmaps, projection_matrices, confidences, H_img, W_img):
    scale_x = float(H_img) / HH
    scale_y = float(W_img) / WH
    nc = _get_nc(scale_x, scale_y)
    in_maps = make_in_maps(heatmaps, projection_matrices, confidences)
    try:
        res = run_bass_kernel_spmd(nc, in_maps, core_ids=list(range(NCORES)))
    except ModuleNotFoundError:
        # BASS_TRACE set but the axon NTFF hook isn't available in this
        # environment -- rerun with tracing force-disabled.
        import os
        os.environ["BASS_NEVER_TRACE"] = "1"
        res = run_bass_kernel_spmd(nc, in_maps, core_ids=list(range(NCORES)))
    return assemble_out(res.results)



# revision 3
# speedup vs baseline: 1.0758x; 1.0758x over previous
"""Trainium2 Bass kernel: AlgebraicTriangulation (softargmax keypoints + DLT).

Contract: kernel(**inputs) takes FULL inputs (as in reference.setup_inputs())
and returns the FULL (64, 17, 3) output. Internally shards batch across 8
NeuronCores (pure data parallel), runs one Bass/Tile program per core via
run_bass_kernel_spmd, and reassembles on host.

Algorithm per core (8 batches = 544 heatmap rows of 96x96, 2176 quarter
units of 2304 px):
  Phase A (streamed over 17 tiles of [128 units, 2304]):
    - DVE Max gives the per-quarter max (stabilizer).
    - ACT computes w = bf16(exp(100*(h - max))) with a per-partition bias.
    - PE transposes w in 128-px chunks into PSUM; chunks are copied back to
      SBUF (DVE+ACT) and matmul'd against constant [1, x, y] stationaries,
      accumulating [Sw, Swx, Swy] per unit in PSUM. No MaxIndex / top-8
      candidate machinery; the whole softargmax reduction runs on PE.
  Phase B: per-row combine of the 4 quarter sums with exp(100*(mq - mrow))
    factors, rescale -> keypoints [68, 8, 2] (DRAM-roundtrip regroup).
  Phase D: DLT via M = A^T A per (batch, joint) and 7 rounds (2 1/3 sweeps)
    of parallel-pairs cyclic Jacobi; eigenvector of the smallest eigenvalue,
    dehomogenize.
"""
import numpy as np
from contextlib import ExitStack

import ml_dtypes

import concourse.bass as bass
import concourse.tile as tile
from concourse import bacc, mybir
from concourse.bass_utils import run_bass_kernel_spmd

f32 = mybir.dt.float32
bf16 = mybir.dt.bfloat16
u32 = mybir.dt.uint32
i32 = mybir.dt.int32
AF = mybir.ActivationFunctionType
ALU = mybir.AluOpType
AX = mybir.AxisListType

B, C, J, HH, WH = 64, 4, 17, 96, 96
NCORES = 8
BB = B // NCORES            # 8 local batches per core
R = BB * C * J              # 544 heatmap rows per core
NQ = 4                      # quarters per heatmap
QF = (HH * WH) // NQ        # 2304
QH = HH // NQ               # 24 rows of the heatmap per quarter
U = R * NQ                  # 2176 quarter-units
NT = U // 128               # 17 tiles
NCH = QF // 128             # 18 pixel chunks per quarter
P2 = (BB // 2) * J          # 68 partitions for row/DLT stages
S8 = 8                      # fold slots: s = (bb//4)*4 + c
INV_TEMP = 100.0
JROUNDS = 7                 # 2 1/3 sweeps of parallel-pairs cyclic Jacobi

# Jacobi parallel-pair rounds: ((p1,q1),(p2,q2)) disjoint.
ROUNDS = [((0, 1), (2, 3)), ((0, 2), (1, 3)), ((0, 3), (1, 2))]


def _build_nc(scale_x: float, scale_y: float):
    nc = bacc.Bacc(
        "TRN2", target_bir_lowering=False, debug=False,
        enable_asserts=False, num_devices=NCORES,
    )
    hm = nc.dram_tensor("hm", [R, HH * WH], f32, kind="ExternalInput").ap()
    pm = nc.dram_tensor("pm", [P2, 2 * C * 12], f32, kind="ExternalInput").ap()
    cf = nc.dram_tensor("cf", [P2, S8], f32, kind="ExternalInput").ap()
    ident = nc.dram_tensor("ident", [128, 128], bf16, kind="ExternalInput").ap()
    coords = nc.dram_tensor("coords", [128, NCH, 3], bf16,
                            kind="ExternalInput").ap()
    qv = nc.dram_tensor("qv", [P2, S8, NQ], f32, kind="ExternalInput").ap()
    out = nc.dram_tensor("out", [P2, 2, 3], f32, kind="ExternalOutput").ap()

    with tile.TileContext(nc) as tc, ExitStack() as ctx:
        _body(ctx, tc, nc, hm, pm, cf, ident, coords, qv, out,
              scale_x, scale_y)
    nc.compile()
    return nc


def _body(ctx, tc, nc, hm, pm, cf, ident_d, coords_d, qv_d, out,
          scale_x, scale_y):
    xpool = ctx.enter_context(tc.tile_pool(name="x", bufs=4))
    wpool = ctx.enter_context(tc.tile_pool(name="w", bufs=2))
    pers = ctx.enter_context(tc.tile_pool(name="pers", bufs=1))
    tmp = ctx.enter_context(tc.tile_pool(name="tmp", bufs=2))
    ps = ctx.enter_context(tc.tile_pool(name="ps", bufs=2, space="PSUM"))
    dpool = ctx.enter_context(tc.tile_pool(name="scratch", bufs=1, space="DRAM"))

    # ---------------- constants ----------------
    ident = pers.tile([128, 128], bf16, tag="ident", name="ident")
    nc.sync.dma_start(ident[:], ident_d)
    coords = pers.tile([128, NCH, 3], bf16, tag="coords", name="coords")
    nc.sync.dma_start(coords[:], coords_d)
    qvt = pers.tile([P2, S8, NQ], f32, tag="qvt", name="qvt")
    nc.sync.dma_start(qvt[:], qv_d)

    # ---------------- Phase A ----------------
    units = hm.rearrange("r (a b) -> (r a) b", a=NQ)          # (2176, 2304)
    sv = pers.tile([128, NT, 8], f32, tag="sv", name="sv")    # top-8 values
    spo_d = dpool.tile([3, U], f32, tag="spo_d", name="spo_d")
    mq_d = dpool.tile([U, 1], f32, tag="mq_d", name="mq_d")
    CA, CB = 8, 16            # chunk split: [0,8) DVE copy, [8,16)+[16,18) ACT
    for t in range(NT):
        x = xpool.tile([128, QF], f32, tag="x", name="x")
        nc.sync.dma_start(x[:], units[t * 128:(t + 1) * 128, :])
        nc.vector.max(sv[:, t, :], x[:])
        bias_t = tmp.tile([128, 1], f32, tag="bias", name="bias")
        nc.vector.tensor_scalar(bias_t[:], sv[:, t, 0:1], -INV_TEMP, None,
                                op0=ALU.mult)
        w = wpool.tile([128, QF], bf16, tag="w", name="w")
        nc.scalar.activation(w[:], x[:], AF.Exp, bias=bias_t[:],
                             scale=INV_TEMP)
        nc.sync.dma_start(mq_d[:][128 * t:128 * (t + 1), :], sv[:, t, 0:1])

        ptA = ps.tile([128, CA * 128], bf16, tag="ptA", name="ptA")
        ptB = ps.tile([128, (CB - CA) * 128], bf16, tag="ptB", name="ptB")
        ptC = ps.tile([128, (NCH - CB) * 128], bf16, tag="ptC", name="ptC")
        for k in range(NCH):
            if k < CA:
                dst = ptA[:, 128 * k:128 * (k + 1)]
            elif k < CB:
                dst = ptB[:, 128 * (k - CA):128 * (k - CA + 1)]
            else:
                dst = ptC[:, 128 * (k - CB):128 * (k - CB + 1)]
            nc.tensor.transpose(dst, w[:, 128 * k:128 * (k + 1)], ident[:])
        wTa = wpool.tile([128, CA * 128], bf16, tag="wTa", name="wTa")
        nc.vector.tensor_copy(wTa[:], ptA[:])
        wTb = wpool.tile([128, (CB - CA) * 128], bf16, tag="wTb", name="wTb")
        nc.scalar.copy(wTb[:], ptB[:])
        wTc = wpool.tile([128, (NCH - CB) * 128], bf16, tag="wTc", name="wTc")
        nc.scalar.copy(wTc[:], ptC[:])

        po = ps.tile([3, 128], f32, tag="po", name="po")
        for k in range(NCH):
            if k < CA:
                src = wTa[:, 128 * k:128 * (k + 1)]
            elif k < CB:
                src = wTb[:, 128 * (k - CA):128 * (k - CA + 1)]
            else:
                src = wTc[:, 128 * (k - CB):128 * (k - CB + 1)]
            nc.tensor.matmul(po[:], coords[:, k, :], src,
                             start=(k == 0), stop=(k == NCH - 1))
        spo = tmp.tile([3, 128], f32, tag="spo", name="spo")
        nc.scalar.copy(spo[:], po[:])
        nc.sync.dma_start(spo_d[:][:, 128 * t:128 * (t + 1)], spo[:])

    # ---------------- Phase B: per-row combine -> keypoints ----------------
    # unit u = 272*s + 4*p' + q  (host row order r = s*68 + p')
    mv2 = pers.tile([P2, 3, S8, NQ], f32, tag="mv2", name="mv2")
    nc.sync.dma_start(mv2[:], spo_d[:].rearrange("i (s p q) -> p i s q",
                                                 s=S8, p=P2))
    mmu = pers.tile([P2, S8, NQ], f32, tag="mmu", name="mmu")
    nc.sync.dma_start(mmu[:], mq_d[:].rearrange("(s p q) one -> p s (q one)",
                                                s=S8, p=P2))

    SHQ = [P2, S8, NQ]

    def T(tag, shape=None, dtype=f32, pool=tmp):
        return pool.tile(shape or SHQ, dtype, tag=tag, name=tag)

    m_r = T("m_r", [P2, S8])
    nc.vector.tensor_reduce(m_r[:], mmu[:], axis=AX.X, op=ALU.max)
    df = T("df")
    nc.any.tensor_sub(df[:], mmu[:], m_r[:].unsqueeze(2).broadcast_to(SHQ))
    fq = T("fq")
    nc.scalar.activation(fq[:], df[:], AF.Exp, bias=0.0, scale=INV_TEMP)
    SH3 = [P2, 3, S8, NQ]
    Sg = T("Sg", SH3)
    nc.any.tensor_mul(Sg[:], mv2[:], fq[:].unsqueeze(1).broadcast_to(SH3))
    Ss = T("Ss", [P2, 3, S8])
    nc.vector.tensor_reduce(Ss[:], Sg[:], axis=AX.X, op=ALU.add)
    # y offset: ky_num += sum_q (QH*q) * fq * Sw   (qv = QH*q from host)
    qfq = T("qfq")
    nc.any.tensor_mul(qfq[:], fq[:], qvt[:])
    qfs = T("qfs")
    nc.any.tensor_mul(qfs[:], qfq[:], mv2[:, 0, :, :])
    Sq = T("Sq", [P2, S8])
    nc.vector.tensor_reduce(Sq[:], qfs[:], axis=AX.X, op=ALU.add)
    rden = T("rden", [P2, S8])
    nc.vector.reciprocal(rden[:], Ss[:, 0, :])
    kpt = pers.tile([P2, S8, 2], f32, tag="kpt", name="kpt")
    nc.vector.scalar_tensor_tensor(kpt[:, :, 0], Ss[:, 1, :], scale_x, rden[:],
                                   op0=ALU.mult, op1=ALU.mult)
    tmpy = T("tmpy", [P2, S8])
    nc.any.tensor_add(tmpy[:], Ss[:, 2, :], Sq[:])
    nc.vector.scalar_tensor_tensor(kpt[:, :, 1], tmpy[:], scale_y, rden[:],
                                   op0=ALU.mult, op1=ALU.mult)

    # ---------------- Phase D: DLT ----------------
    pmt = pers.tile([P2, 2 * C * 12], f32, tag="pmt", name="pmt")
    nc.sync.dma_start(pmt[:], pm)
    cft = pers.tile([P2, S8], f32, tag="cft", name="cft")
    nc.sync.dma_start(cft[:], cf)

    # (b, c) merged into one free dim bc = b*4+c so every AP has <= 3 free dims
    pmr = pmt[:].rearrange("p (bc r k) -> p bc r k", r=3, k=4)              # (68,8,3,4)
    ASH = [P2, 2 * C, 2, 4]
    a0 = T("a0", ASH)
    nc.any.tensor_mul(a0[:], pmr[:, :, 2, :].unsqueeze(2).broadcast_to(ASH),
                      kpt[:].unsqueeze(3).broadcast_to(ASH))
    a1 = T("a1", ASH)
    nc.any.tensor_sub(a1[:], a0[:], pmr[:, :, 0:2, :])
    At = T("At", ASH)
    nc.any.tensor_mul(At[:], a1[:],
                      cft[:].unsqueeze(2).unsqueeze(3).broadcast_to(ASH))

    # M = A^T A per b-slot: prod[p,k,l,r] = A[p,r,k]*A[p,r,l], reduce over r
    Mt = pers.tile([P2, 2, 4, 4], f32, tag="Mt", name="Mt")
    for b in range(2):
        Ab = At[:, C * b:C * (b + 1), :, :].rearrange("p c i k -> p (c i) k")
        Akr = Ab.transpose([0, 2, 1])                                  # (68,4,8)
        PSH = [P2, 4, 4, 8]
        prod = T(f"prod{b}", PSH)
        nc.any.tensor_mul(prod[:], Akr.unsqueeze(2).broadcast_to(PSH),
                          Akr.unsqueeze(1).broadcast_to(PSH))
        nc.vector.tensor_reduce(Mt[:, b, :, :], prod[:], axis=AX.X, op=ALU.add)

    Mf = Mt[:].rearrange("p b k l -> p b (k l)")                       # (68,2,16)
    tr_ = T("tr", [P2, 2])
    nc.vector.tensor_reduce(tr_[:], Mf[:, :, 0:16:5], axis=AX.X, op=ALU.add)
    rtr = T("rtr", [P2, 2])
    nc.vector.reciprocal(rtr[:], tr_[:])
    Mn = pers.tile([P2, 2, 4, 4], f32, tag="Mn")
    nc.any.tensor_mul(Mn[:].rearrange("p b k l -> p b (k l)"), Mf,
                      rtr[:].unsqueeze(2).broadcast_to([P2, 2, 16]))

    eps_t = pers.tile([P2, 1], f32, tag="eps_t", name="eps_t")
    nc.vector.memset(eps_t[:], 1e-35)

    Vt = pers.tile([P2, 2, 4, 4], f32, tag="Vt")
    Vf = Vt[:].rearrange("p b i k -> p b (i k)")
    nc.vector.memset(Vt[:], 0.0)
    nc.vector.memset(Vf[:, :, 0:16:5], 1.0)

    Mnf = Mn[:].rearrange("p b k l -> p b (k l)")
    ASH2 = [P2, 2, 2]
    BSH = [P2, 2, 2, 4]

    def jacobi_round(rp, last=False, fillers=(None, None)):
        (p1, q1), (p2, q2) = rp
        dg = lambda k: 5 * k
        of = lambda k, l: 4 * k + l

        def sl2(a, b):
            st = b - a
            if st > 0:
                return Mnf[:, :, a:b + 1:st]
            return Mnf[:, :, a:b - 1:st]

        app = sl2(dg(p1), dg(p2))
        aqq = sl2(dg(q1), dg(q2))
        apq = sl2(of(p1, q1), of(p2, q2))

        dd = T("j_dd", ASH2)
        nc.any.tensor_sub(dd[:], aqq, app)
        aq2 = T("j_aq2", ASH2)
        nc.any.tensor_mul(aq2[:], apq, apq)
        d2 = T("j_d2", ASH2)
        nc.any.tensor_mul(d2[:], dd[:], dd[:])
        sg = T("j_sg", ASH2)
        nc.vector.tensor_scalar(sg[:], dd[:], 0.0, None, op0=ALU.is_ge)
        sg1 = T("j_sg1", ASH2)
        nc.vector.tensor_scalar(sg1[:], sg[:], 2.0, -1.0, op0=ALU.mult, op1=ALU.add)
        h2 = T("j_h2", ASH2)
        nc.vector.scalar_tensor_tensor(h2[:], aq2[:], 4.0, d2[:],
                                       op0=ALU.mult, op1=ALU.add)
        # h = sqrt(dd^2 + 4 apq^2 + eps); eps via the sqrt bias keeps h > 0
        hh = T("j_hh", ASH2)
        nc.scalar.activation(hh[:], h2[:], AF.Sqrt, bias=eps_t[:])
        ab = T("j_ab", ASH2)
        nc.any.tensor_mul(ab[:], sg1[:], dd[:])
        if fillers[0] is not None:
            fillers[0]()          # fills the DVE wait on the ACT sqrt
        rh = T("j_rh", ASH2)
        nc.vector.reciprocal(rh[:], hh[:])
        gam = T("j_gam", ASH2)
        nc.any.tensor_mul(gam[:], ab[:], rh[:])
        s1 = T("j_s1", ASH2)
        nc.any.tensor_mul(s1[:], apq, rh[:])
        cc2 = T("j_cc2", ASH2)
        nc.vector.tensor_scalar(cc2[:], gam[:], 0.5, 0.5, op0=ALU.mult, op1=ALU.add)
        s2 = T("j_s2", ASH2)
        nc.any.tensor_mul(s2[:], s1[:], sg1[:])
        # c = sqrt((1+gamma)/2); s = (apq/h)/c * sign(dd):  c^2+s^2 == 1 to
        # machine precision because s inherits 1/c (orthogonality-safe even
        # with a sloppy ACT sqrt).
        cC = T("j_c", ASH2)
        nc.scalar.sqrt(cC[:], cc2[:])
        if fillers[1] is not None:
            fillers[1]()          # fills the DVE wait on the ACT sqrt
        rc = T("j_rc", ASH2)
        nc.vector.reciprocal(rc[:], cC[:])
        sS = T("j_s", ASH2)
        nc.any.tensor_mul(sS[:], s2[:], rc[:])

        ccb = cC[:].unsqueeze(3).broadcast_to(BSH)
        ssb = sS[:].unsqueeze(3).broadcast_to(BSH)

        def rows_view(base, i1, i2):
            st = i2 - i1
            if st > 0:
                return base[:, :, i1:i2 + 1:st, :]
            return base[:, :, i1:i2 - 1:st, :]

        def rotate(base, tagp, eng):
            rP = rows_view(base, p1, p2)
            rQ = rows_view(base, q1, q2)
            t1 = T(tagp + "_1", BSH)
            eng.tensor_mul(t1[:], ccb, rP)
            t2 = T(tagp + "_2", BSH)
            eng.tensor_mul(t2[:], ssb, rQ)
            t3 = T(tagp + "_3", BSH)
            eng.tensor_mul(t3[:], ssb, rP)
            t4 = T(tagp + "_4", BSH)
            eng.tensor_mul(t4[:], ccb, rQ)
            eng.tensor_sub(rP, t1[:], t2[:])
            eng.tensor_add(rQ, t3[:], t4[:])

        if last:
            # only M's diagonal is consumed after the final rotation:
            # app' = app - t*apq, aqq' = aqq + t*apq (exact), t = s/c
            t_ = T("j_t", ASH2)
            nc.any.tensor_mul(t_[:], sS[:], rc[:])
            tapq = T("j_tapq", ASH2)
            nc.any.tensor_mul(tapq[:], t_[:], apq)
            nc.any.tensor_sub(app, app, tapq[:])
            nc.any.tensor_add(aqq, aqq, tapq[:])
        else:
            rotate(Mn[:], "j_r", nc.any)                           # rows
            rotate(Mn[:].transpose([0, 1, 3, 2]), "j_c_", nc.any)  # cols
        # V-col rotation split into two closures: the caller emits them one
        # round later, at the next round's ACT-sqrt wait points (they only
        # depend on this round's c, s).
        base = Vt[:].transpose([0, 1, 3, 2])
        rP = rows_view(base, p1, p2)
        rQ = rows_view(base, q1, q2)

        def vrot_a():
            t1 = T("j_v_1", BSH)
            nc.any.tensor_mul(t1[:], ccb, rP)
            t2 = T("j_v_2", BSH)
            nc.any.tensor_mul(t2[:], ssb, rQ)
            t3 = T("j_v_3", BSH)
            nc.any.tensor_mul(t3[:], ssb, rP)
            nc.any.tensor_sub(rP, t1[:], t2[:])
            vrot_a.t3 = t3

        def vrot_b():
            t4 = T("j_v_4", BSH)
            nc.any.tensor_mul(t4[:], ccb, rQ)
            nc.any.tensor_add(rQ, vrot_a.t3[:], t4[:])
        return (vrot_a, vrot_b)

    pending_v = (None, None)
    for ri in range(JROUNDS):
        pending_v = jacobi_round(
            ROUNDS[ri % 3], last=(ri == JROUNDS - 1), fillers=pending_v)
    pending_v[0]()
    pending_v[1]()

    # ---------------- extract smallest-eigenvalue column, dehomogenize ----------------
    diag = Mnf[:, :, 0:16:5]
    dmin = T("dmin", [P2, 2])
    nc.vector.tensor_reduce(dmin[:], diag, axis=AX.X, op=ALU.min)
    maskm = T("maskm", [P2, 2, 4])
    nc.any.tensor_tensor(maskm[:], diag, dmin[:].unsqueeze(2).broadcast_to([P2, 2, 4]),
                         op=ALU.is_equal)
    vm = T("vm", [P2, 2, 4, 4])
    nc.any.tensor_mul(vm[:], Vt[:], maskm[:].unsqueeze(2).broadcast_to([P2, 2, 4, 4]))
    vsel = T("vsel", [P2, 2, 4])
    nc.vector.tensor_reduce(vsel[:], vm[:], axis=AX.X, op=ALU.add)
    rw = T("rw", [P2, 2])
    nc.vector.reciprocal(rw[:], vsel[:, :, 3])
    o3 = T("o3", [P2, 2, 3])
    nc.any.tensor_mul(o3[:], vsel[:, :, 0:3],
                      rw[:].unsqueeze(2).broadcast_to([P2, 2, 3]))
    nc.sync.dma_start(out, o3[:])


_NC_CACHE = {}


def _get_nc(scale_x: float, scale_y: float):
    key = (scale_x, scale_y)
    if key not in _NC_CACHE:
        _NC_CACHE[key] = _build_nc(scale_x, scale_y)
    return _NC_CACHE[key]


def _aux_arrays():
    ident = np.eye(128, dtype=ml_dtypes.bfloat16)
    pix = np.arange(QF)
    coords = np.stack(
        [np.ones(QF), pix % WH, pix // WH], axis=-1
    ).reshape(NCH, 128, 3).transpose(1, 0, 2).astype(ml_dtypes.bfloat16)
    qv = np.broadcast_to((QH * np.arange(NQ)).astype(np.float32),
                         (P2, S8, NQ)).copy()
    return ident, coords, qv


def make_in_maps(heatmaps, projection_matrices, confidences):
    hm = np.asarray(heatmaps, dtype=np.float32)
    P = np.asarray(projection_matrices, dtype=np.float32)
    cfa = np.asarray(confidences, dtype=np.float32)
    hmv = hm.reshape(NCORES, 2, BB // 2, C, J, HH * WH)   # (core, bb2, bb4, c, j, F)
    Pv = P.reshape(NCORES, 2, BB // 2, C, 3, 4)
    cfv = cfa.reshape(NCORES, 2, BB // 2, C, J)
    ident, coords, qv = _aux_arrays()
    in_maps = []
    for k in range(NCORES):
        hmk = np.ascontiguousarray(hmv[k].transpose(0, 2, 1, 3, 4)).reshape(R, HH * WH)
        pmk = np.ascontiguousarray(np.broadcast_to(
            Pv[k].transpose(1, 0, 2, 3, 4)[:, None], (BB // 2, J, 2, C, 3, 4)
        )).reshape(P2, 2 * C * 12)
        cfk = np.ascontiguousarray(cfv[k].transpose(1, 3, 0, 2)).reshape(P2, S8)
        in_maps.append({"hm": hmk, "pm": pmk, "cf": cfk,
                        "ident": ident, "coords": coords, "qv": qv})
    return in_maps


def assemble_out(results):
    outs = []
    for k in range(NCORES):
        o = results[k]["out"].reshape(BB // 2, J, 2, 3).transpose(2, 0, 1, 3)
        outs.append(o.reshape(BB, J, 3))
    return np.concatenate(outs, axis=0)


def kernel(heatmaps, projection_matrices, confidences, H_img, W_img):
    scale_x = float(H_img) / HH
    scale_y = float(W_img) / WH
    nc = _get_nc(scale_x, scale_y)
    in_maps = make_in_maps(heatmaps, projection_matrices, confidences)
    try:
        res = run_bass_kernel_spmd(nc, in_maps, core_ids=list(range(NCORES)))
    except ModuleNotFoundError:
        # BASS_TRACE set but the axon NTFF hook isn't available in this
        # environment -- rerun with tracing force-disabled.
        import os
        os.environ["BASS_NEVER_TRACE"] = "1"
        res = run_bass_kernel_spmd(nc, in_maps, core_ids=list(range(NCORES)))
    return assemble_out(res.results)
